# revision 19
# baseline (speedup 1.0000x reference)
"""Causal single-head attention (B=4, S=2048, D=1024) on 8 Trainium2 NeuronCores.

Sharding: core = (batch, parity). Each batch's 8 query-chunks of 256 are split
{0,3,4,7} / {1,2,5,6} across its two cores so causal work balances exactly.

Algebraic restructure vs the naive QKV form (saves 40% of PE work):
  scores = (x Wq^T)(x Wk^T)^T = x (Wq^T Wk) x^T = x M x^T
with M = Wq^T Wk / 32 precomputed on the HOST (weights-only precompute) — the
K projection disappears and scores contract q' = x_q M directly against raw
x^T. Likewise
  out = P v = P (x Wv^T) = (P x) Wv^T
so the V projection disappears and P contracts against raw x; the small
(Px) Wv^T projection runs once per query block. Per-core PE work drops from
8.05G to 4.83G MACs with no cross-core communication.

All big matmuls are bf16 (fp8 DoubleRow's 2x was measured to cost ~1.5e-2
output error from quantizing q'/keys — too close to the 2e-2 budget). Only
the softmax denominator runs fp8-DR (its quantization error averages down
over keys; this halves ldweights pressure vs 240 tiny bf16 matmuls).

Device algorithm per core (fp32 PSUM accumulation):
  q'T[j,q]   = m_t.T @ xTq           bf16   (1.07G)  512-wide groups
  s_T[sk,sq] = xT_blk.T @ q'T        bf16   (1.34G)  logits direct (1/32 in M)
  p = exp(s_T) * mask                (no max-subtraction: logits are O(1))
  den[sq]    = p8_blk.T @ ones       fp8-DR
  PxT[i,sq]  = xS_blk.T @ p_blk      bf16   (1.34G)  L-pair-merged rhs
  out[sq,o]  = (PxT.T @ WvT) / den   bf16   (1.07G)
"""

import sys

if "/opt/trn_rl_repo" not in sys.path:
    sys.path.insert(0, "/opt/trn_rl_repo")

import numpy as np
import ml_dtypes

import concourse.mybir as mybir
import concourse.tile as tile
from concourse import bacc
from concourse.bass_utils import run_bass_kernel_spmd

bf16 = ml_dtypes.bfloat16
f8 = ml_dtypes.float8_e4m3

B, S, D = 4, 2048, 1024
CH = 512            # xT column-chunk width (SBUF tile granularity)
QC = 256            # query-chunk width
BLK = 128           # key-block
# Per-core schedule: 4 query-chunks of 256, processed with a fixed padded
# k-block count (4,8,12,16). Host assigns real chunks sorted by causal depth
# so padding waste is exactly 4 blocks/core; masks (data) encode reality.
SCHED = (4, 8, 12, 16)
MASK_BASE = (0, 4, 12, 24)
NMASK = sum(SCHED)  # 40
DT8 = D // 128      # contraction tiles
N_CORES = 8
DT_BF = mybir.dt.bfloat16
DT_F8 = mybir.dt.float8e4
DT_F32 = mybir.dt.float32
DR = mybir.MatmulPerfMode.DoubleRow

_NC_CACHE = {}


def _emit(tc, xT, xTq, xS, m, wvT, msk, out):
    nc = tc.nc
    Exp = mybir.ActivationFunctionType.Exp
    Copy = mybir.ActivationFunctionType.Copy

    with (
        tc.tile_pool(name="const", bufs=1) as constp,
        tc.tile_pool(name="sb", bufs=1) as sb,
        tc.tile_pool(name="outs_sb", bufs=2) as osb,
        tc.tile_pool(name="es_sb", bufs=4) as esb,
        tc.tile_pool(name="qps", bufs=1, space="PSUM") as qps,
        tc.tile_pool(name="sps", bufs=2, space="PSUM") as sps,
        tc.tile_pool(name="pxps", bufs=2, space="PSUM") as pxps,
        tc.tile_pool(name="ops", bufs=1, space="PSUM") as ops,
        tc.tile_pool(name="dps", bufs=1, space="PSUM") as dps,
    ):
        ones8 = constp.tile([128, 2, 1], DT_F8, tag="ones8", name="ones8")
        nc.vector.memset(ones8, 1.0)
        ones = constp.tile([128, 1], DT_BF, tag="ones", name="ones")
        nc.vector.memset(ones, 1.0)

        m_t = [
            [sb.tile([128, 128], DT_BF, tag=f"m{i}_{ot}", name=f"m{i}_{ot}")
             for ot in range(DT8)]
            for i in range(DT8)
        ]
        xTq_t = [
            [sb.tile([128, CH], DT_BF, tag=f"xq{i}_{h}", name=f"xq{i}_{h}")
             for h in range(2)]
            for i in range(DT8)
        ]
        xT_t = [
            [sb.tile([128, CH], DT_BF, tag=f"xT{i}_{sc}", name=f"xT{i}_{sc}")
             for sc in range(S // CH)]
            for i in range(DT8)
        ]
        xS_t = [sb.tile([128, D], DT_BF, tag=f"xS{b}", name=f"xS{b}")
                for b in range(S // 128)]
        wv_t = [sb.tile([128, D], DT_BF, tag=f"wv{i}", name=f"wv{i}")
                for i in range(DT8)]
        qT_t = [sb.tile([128, 4 * QC], DT_BF, tag=f"qT{i}", name=f"qT{i}")
                for i in range(DT8)]
        msk_t = [sb.tile([128, QC], DT_BF, tag=f"msk{n}", name=f"msk{n}")
                 for n in range(NMASK)]
        # p tiles pair two L-chunks side by side (cols 0:256 = even L,
        # 256:512 = odd L) so one Px matmul covers both while the shared
        # k-blocks last; beyond the even L's schedule only cols 256:512 live.
        p01_t = [sb.tile([128, 2 * QC], DT_BF, tag=f"p01_{b}", name=f"p01_{b}")
                 for b in range(SCHED[1])]
        p23_t = [sb.tile([128, 2 * QC], DT_BF, tag=f"p23_{b}", name=f"p23_{b}")
                 for b in range(SCHED[3])]
        # fp8 copy of p for the DoubleRow denominator matmul
        p8_t = [sb.tile([128, SCHED[L], QC], DT_F8, tag=f"p8_{L}", name=f"p8_{L}")
                for L in range(4)]
        px_t = {}
        for L in range(4):
            for i in range(DT8):
                px_t[(L, i)] = sb.tile([128, QC], DT_BF, tag=f"px{L}_{i}",
                                       name=f"px{L}_{i}")

        def p_slice(L, b):
            t = p01_t if L < 2 else p23_t
            return t[b][:, QC * (L % 2) : QC * (L % 2 + 1)]

        # Input DMAs ride sync+gpsimd only (scalar/vector run the
        # latency-critical softmax pipeline; an engine's in-order queue would
        # stall it behind queued DMA triggers), in consumption order so the
        # first matmuls start early.
        _dmaq = [nc.sync, nc.gpsimd]

        def _dma(n, dst, src):
            _dmaq[n % 2].dma_start(out=dst, in_=src)

        def dma_m(ot):
            for i in range(DT8):
                _dma(i + ot, m_t[i][ot],
                     m[128 * i : 128 * (i + 1), 128 * ot : 128 * (ot + 1)])

        def dma_xTq(h):
            for i in range(DT8):
                _dma(i + h, xTq_t[i][h],
                     xTq[128 * i : 128 * (i + 1), CH * h : CH * (h + 1)])

        def dma_xT(sc):
            for i in range(DT8):
                _dma(i + sc, xT_t[i][sc],
                     xT[128 * i : 128 * (i + 1), CH * sc : CH * (sc + 1)])

        def dma_xS(b0, b1):
            for b in range(b0, b1):
                _dma(b, xS_t[b], xS[128 * b : 128 * (b + 1), :])

        def dma_msk(L):
            for b in range(SCHED[L]):
                _dma(b, msk_t[MASK_BASE[L] + b], msk[MASK_BASE[L] + b])

        def dma_wv():
            for i in range(DT8):
                _dma(i + 1, wv_t[i], wvT[128 * i : 128 * (i + 1), :])

        dma_m(0)
        dma_xTq(0)
        for ot in range(1, DT8):
            dma_m(ot)
        dma_xT(0)
        dma_msk(0)
        dma_xT(1)
        dma_msk(1)
        dma_xTq(1)
        dma_xS(0, 8)
        dma_xT(2)
        dma_msk(2)
        dma_wv()
        dma_xS(8, 12)
        dma_xT(3)
        dma_msk(3)
        dma_xS(12, 16)

        # ---- PE pipeline ----
        def q_proj(h):
            # q'T[j, q] for the L-pair (2h, 2h+1): 512-wide rhs covers both
            for ot in range(DT8):
                ps = qps.tile([128, CH], DT_F32, tag="qps", name="qps")
                for i in range(DT8):
                    nc.tensor.matmul(
                        ps,
                        lhsT=m_t[i][ot],
                        rhs=xTq_t[i][h],
                        start=(i == 0),
                        stop=(i == DT8 - 1),
                    )
                nc.scalar.copy(out=qT_t[ot][:, CH * h : CH * (h + 1)], in_=ps)

        def scores(L):
            for b in range(SCHED[L]):
                ps = sps.tile([128, QC], DT_F32, tag="sps", name="sps")
                for i in range(DT8):
                    nc.tensor.matmul(
                        ps,
                        lhsT=xT_t[i][b // 4][:, BLK * (b % 4) : BLK * (b % 4 + 1)],
                        rhs=qT_t[i][:, QC * L : QC * (L + 1)],
                        start=(i == 0),
                        stop=(i == DT8 - 1),
                    )
                es = esb.tile([128, QC], DT_BF, tag="es", name="es")
                nc.scalar.activation(es, ps, Exp)
                mk = msk_t[MASK_BASE[L] + b]
                nc.vector.tensor_mul(p_slice(L, b), es, mk)
                if L > 0:
                    nc.vector.tensor_mul(p8_t[L][:, b, :], es, mk)

        def px(Le):
            # PxT[i, q] for the L-pair (Le, Le+1): one matmul covers both
            # chunks' q-columns while b < SCHED[Le]; beyond that only the
            # odd chunk's half accumulates.
            nb_e, nb_o = SCHED[Le], SCHED[Le + 1]
            pt = p01_t if Le == 0 else p23_t
            for i in range(DT8):
                ps = pxps.tile([128, 2 * QC], DT_F32, tag="pxps", name="pxps")
                for b in range(nb_o):
                    w = xS_t[b][:, 128 * i : 128 * (i + 1)]
                    if b < nb_e:
                        nc.tensor.matmul(
                            ps, lhsT=w, rhs=pt[b],
                            start=(b == 0), stop=(b == nb_o - 1),
                            skip_group_check=True,
                        )
                    else:
                        nc.tensor.matmul(
                            ps[:, QC : 2 * QC], lhsT=w, rhs=pt[b][:, QC : 2 * QC],
                            start=False, stop=(b == nb_o - 1),
                            skip_group_check=True,
                        )
                if i % 2 == 0:
                    nc.scalar.copy(out=px_t[(Le, i)], in_=ps[:, 0:QC])
                    nc.vector.tensor_copy(out=px_t[(Le + 1, i)], in_=ps[:, QC : 2 * QC])
                else:
                    nc.vector.tensor_copy(out=px_t[(Le, i)], in_=ps[:, 0:QC])
                    nc.scalar.copy(out=px_t[(Le + 1, i)], in_=ps[:, QC : 2 * QC])

        def den_out(L):
            for sqt in range(QC // 128):
                pd = dps.tile([128, 1], DT_F32, tag="pd", name="pd")
                if L == 0:
                    # exact bf16 den for the shortest chunk: its first rows
                    # see too few keys for the fp8 den error to average out
                    for b in range(SCHED[0]):
                        nc.tensor.matmul(
                            pd,
                            lhsT=p_slice(0, b)[:, 128 * sqt : 128 * (sqt + 1)],
                            rhs=ones,
                            start=(b == 0),
                            stop=(b == SCHED[0] - 1),
                        )
                else:
                    for u in range(SCHED[L] // 2):
                        nc.tensor.matmul(
                            pd,
                            lhsT=p8_t[L][:, 2 * u : 2 * u + 2,
                                         128 * sqt : 128 * (sqt + 1)],
                            rhs=ones8,
                            perf_mode=DR,
                            start=(u == 0),
                            stop=(u == SCHED[L] // 2 - 1),
                        )
                po = ops.tile([128, D], DT_F32, tag="po", name="po")
                for i in range(DT8):
                    pxs = px_t[(L, i)][:, 128 * sqt : 128 * (sqt + 1)]
                    nc.tensor.matmul(
                        po[:, 0:CH], lhsT=pxs, rhs=wv_t[i][:, 0:CH],
                        start=(i == 0), stop=(i == DT8 - 1),
                        skip_group_check=True,
                    )
                    nc.tensor.matmul(
                        po[:, CH:D], lhsT=pxs, rhs=wv_t[i][:, CH:D],
                        start=(i == 0), stop=(i == DT8 - 1),
                        skip_group_check=True,
                    )
                r = osb.tile([128, 1], DT_F32, tag="r", name="r")
                nc.vector.reciprocal(r, pd)
                o = osb.tile([128, D], DT_BF, tag="osb", name="osb")
                if (L + sqt) % 2 == 0:
                    nc.scalar.activation(o, po, Copy, scale=r)
                else:
                    nc.vector.tensor_scalar_mul(o, po, r)
                nc.sync.dma_start(
                    out=out[QC * L + 128 * sqt : QC * L + 128 * (sqt + 1), :],
                    in_=o,
                )

        q_proj(0)
        scores(0)
        scores(1)
        q_proj(1)
        scores(2)
        px(0)
        den_out(0)
        den_out(1)
        scores(3)
        px(2)
        den_out(2)
        den_out(3)


def build_program():
    nc = bacc.Bacc(
        "TRN2",
        target_bir_lowering=False,
        debug=False,
        enable_asserts=False,
        num_devices=N_CORES,
    )
    xT = nc.dram_tensor("xT", [D, S], DT_BF, kind="ExternalInput").ap()
    xTq = nc.dram_tensor("xTq", [D, 4 * QC], DT_BF, kind="ExternalInput").ap()
    xS = nc.dram_tensor("xS", [S, D], DT_BF, kind="ExternalInput").ap()
    m = nc.dram_tensor("m", [D, D], DT_BF, kind="ExternalInput").ap()
    wvT = nc.dram_tensor("wvT", [D, D], DT_BF, kind="ExternalInput").ap()
    msk = nc.dram_tensor("msk", [NMASK, BLK, QC], DT_BF, kind="ExternalInput").ap()
    out = nc.dram_tensor("out", [4 * QC, D], DT_BF, kind="ExternalOutput").ap()
    with tile.TileContext(nc) as tc:
        _emit(tc, xT, xTq, xS, m, wvT, msk, out)
    nc.compile()
    return nc


def get_program():
    if "nc" not in _NC_CACHE:
        _NC_CACHE["nc"] = build_program()
    return _NC_CACHE["nc"]


def _chunks_for(core):
    """Per-core 256-wide query chunks, L-ordered to match SCHED=(4,8,12,16).
    Real causal k-block need: chunk j -> 2(j+1)."""
    return [0, 3, 4, 7] if core % 2 == 0 else [1, 2, 5, 6]


def _build_masks(chunks, permuted):
    """[40,128,256] in {0,1}: allowed iff actual_key <= actual_query, where
    for odd cores the key axis is permuted by pos^256 (see build_in_maps).
    Padding blocks beyond a chunk's real causal depth come out all-zero."""
    m = np.zeros((NMASK, BLK, QC), np.float32)
    p = np.arange(BLK)[:, None]
    c = np.arange(QC)[None, :]
    for L, j in enumerate(chunks):
        for b in range(SCHED[L]):
            sk = BLK * b + p
            if permuted:
                sk = sk ^ 256
            m[MASK_BASE[L] + b] = sk <= QC * j + c
    return m.astype(bf16)


def _perm256(a, axis):
    """Swap the 256-halves of every 512-chunk along `axis` (pos -> pos^256)."""
    sh = a.shape
    n = sh[axis]
    new_shape = sh[:axis] + (n // 512, 2, 256) + sh[axis + 1 :]
    return np.ascontiguousarray(
        np.flip(a.reshape(new_shape), axis=axis + 1).reshape(sh)
    )


def build_in_maps(x, Wq, Wk, Wv):
    Wq = np.asarray(Wq, np.float32)
    Wk = np.asarray(Wk, np.float32)
    Wv = np.asarray(Wv, np.float32)
    m = ((Wq.T @ Wk) / 32.0).astype(bf16)  # [d_in, d_in], softmax scale folded
    wv = np.ascontiguousarray(Wv.T).astype(bf16)
    masks = {par: _build_masks(_chunks_for(par), par == 1) for par in (0, 1)}
    in_maps = []
    for core in range(N_CORES):
        b = core // 2
        xb = np.asarray(x[b], np.float32).astype(bf16)  # [S, D]
        xT = np.ascontiguousarray(np.asarray(x[b], np.float32).T)
        xq = np.concatenate(
            [xT[:, QC * j : QC * (j + 1)] for j in _chunks_for(core)], axis=1
        ).astype(bf16)
        if core % 2 == 1:
            # Key/seq-permute by pos^256 so both parities share one
            # instruction stream; xT columns, xS rows and mask key
            # coordinates move together (q-side xTq is gathered on host and
            # needs no permutation).
            xT = _perm256(xT, 1)
            xb = _perm256(xb, 0)
        in_maps.append(
            {"xT": xT.astype(bf16), "xTq": xq, "xS": xb, "m": m, "wvT": wv,
             "msk": masks[core % 2]}
        )
    return in_maps


def assemble_output(results):
    out = np.zeros((B, S, D), np.float32)
    for core in range(N_CORES):
        b = core // 2
        for L, j in enumerate(_chunks_for(core)):
            out[b, QC * j : QC * (j + 1)] = \
                results[core]["out"][QC * L : QC * (L + 1)].astype(np.float32)
    return out


def kernel(x, Wq, Wk, Wv):
    x = np.asarray(x, np.float32)
    nc = get_program()
    in_maps = build_in_maps(x, np.asarray(Wq, np.float32),
                            np.asarray(Wk, np.float32), np.asarray(Wv, np.float32))
    res = run_bass_kernel_spmd(nc, in_maps, core_ids=list(range(N_CORES)))
    return assemble_output(res.results)


# revision 20
# speedup vs baseline: 1.0436x; 1.0436x over previous
"""Causal single-head attention (B=4, S=2048, D=1024) on 8 Trainium2 NeuronCores.

Sharding: core = (batch, parity). Each batch's 8 query-chunks of 256 are split
{0,3,4,7} / {1,2,5,6} across its two cores so causal work balances exactly.

Algebraic restructure vs the naive QKV form (saves 40% of PE work):
  scores = (x Wq^T)(x Wk^T)^T = x (Wq^T Wk) x^T = x M x^T
with M = Wq^T Wk / 32 precomputed on the HOST (weights-only precompute) — the
K projection disappears and scores contract q' = x_q M directly against raw
x^T. Likewise
  out = P v = P (x Wv^T) = (P x) Wv^T
so the V projection disappears and P contracts against raw x; the small
(Px) Wv^T projection runs once per query block. Per-core PE work drops from
8.05G to 4.83G MACs with no cross-core communication.

All big matmuls are bf16 (fp8 DoubleRow's 2x was measured to cost ~1.5e-2
output error from quantizing q'/keys — too close to the 2e-2 budget). Only
the softmax denominator runs fp8-DR (its quantization error averages down
over keys; this halves ldweights pressure vs 240 tiny bf16 matmuls).

Device algorithm per core (fp32 PSUM accumulation):
  q'T[j,q]   = m_t.T @ xTq           bf16   (1.07G)  512-wide groups
  s_T[sk,sq] = xT_blk.T @ q'T        bf16   (1.34G)  logits direct (1/32 in M)
  p = exp(s_T) * mask                (no max-subtraction: logits are O(1))
  den[sq]    = p8_blk.T @ ones       fp8-DR
  PxT[i,sq]  = xS_blk.T @ p_blk      bf16   (1.34G)  L-pair-merged rhs
  out[sq,o]  = (PxT.T @ WvT) / den   bf16   (1.07G)
"""

import sys

if "/opt/trn_rl_repo" not in sys.path:
    sys.path.insert(0, "/opt/trn_rl_repo")

import numpy as np
import ml_dtypes

import concourse.mybir as mybir
import concourse.tile as tile
from concourse import bacc
from concourse.bass_utils import run_bass_kernel_spmd

bf16 = ml_dtypes.bfloat16
f8 = ml_dtypes.float8_e4m3

B, S, D = 4, 2048, 1024
CH = 512            # xT column-chunk width (SBUF tile granularity)
QC = 256            # query-chunk width
BLK = 128           # key-block
# Per-core schedule: 4 query-chunks of 256, processed with a fixed padded
# k-block count (4,8,12,16). Host assigns real chunks sorted by causal depth
# so padding waste is exactly 4 blocks/core; masks (data) encode reality.
SCHED = (4, 8, 12, 16)
MASK_BASE = (0, 4, 12, 24)
NMASK = sum(SCHED)  # 40
DT8 = D // 128      # contraction tiles
N_CORES = 8
DT_BF = mybir.dt.bfloat16
DT_F8 = mybir.dt.float8e4
DT_F32 = mybir.dt.float32
DR = mybir.MatmulPerfMode.DoubleRow

_NC_CACHE = {}


def _emit(tc, xT, xTq, xS, m, wvT, msk, out):
    nc = tc.nc
    Exp = mybir.ActivationFunctionType.Exp
    Copy = mybir.ActivationFunctionType.Copy

    with (
        tc.tile_pool(name="const", bufs=1) as constp,
        tc.tile_pool(name="sb", bufs=1) as sb,
        tc.tile_pool(name="outs_sb", bufs=2) as osb,
        tc.tile_pool(name="es_sb", bufs=4) as esb,
        tc.tile_pool(name="qps", bufs=2, space="PSUM") as qps,
        tc.tile_pool(name="sps", bufs=2, space="PSUM") as sps,
        tc.tile_pool(name="pxps", bufs=2, space="PSUM") as pxps,
        tc.tile_pool(name="ops", bufs=1, space="PSUM") as ops,
    ):
        ones8 = constp.tile([128, 2, 1], DT_F8, tag="ones8", name="ones8")
        nc.vector.memset(ones8, 1.0)
        ones = constp.tile([128, 1], DT_BF, tag="ones", name="ones")
        nc.vector.memset(ones, 1.0)

        m_t = [
            [sb.tile([128, 128], DT_BF, tag=f"m{i}_{ot}", name=f"m{i}_{ot}")
             for ot in range(DT8)]
            for i in range(DT8)
        ]
        xTq_t = [
            [sb.tile([128, CH], DT_BF, tag=f"xq{i}_{h}", name=f"xq{i}_{h}")
             for h in range(2)]
            for i in range(DT8)
        ]
        xT_t = [
            [sb.tile([128, CH], DT_BF, tag=f"xT{i}_{sc}", name=f"xT{i}_{sc}")
             for sc in range(S // CH)]
            for i in range(DT8)
        ]
        xS_t = [sb.tile([128, D], DT_BF, tag=f"xS{b}", name=f"xS{b}")
                for b in range(S // 128)]
        wv_t = [sb.tile([128, D], DT_BF, tag=f"wv{i}", name=f"wv{i}")
                for i in range(DT8)]
        qT_t = [sb.tile([128, 4 * QC], DT_BF, tag=f"qT{i}", name=f"qT{i}")
                for i in range(DT8)]
        msk_t = [sb.tile([128, QC], DT_BF, tag=f"msk{n}", name=f"msk{n}")
                 for n in range(NMASK)]
        # p tiles pair two L-chunks side by side (cols 0:256 = even L,
        # 256:512 = odd L) so one Px matmul covers both while the shared
        # k-blocks last; beyond the even L's schedule only cols 256:512 live.
        p01_t = [sb.tile([128, 2 * QC], DT_BF, tag=f"p01_{b}", name=f"p01_{b}")
                 for b in range(SCHED[1])]
        p23_t = [sb.tile([128, 2 * QC], DT_BF, tag=f"p23_{b}", name=f"p23_{b}")
                 for b in range(SCHED[3])]
        # fp8 copy of p for the DoubleRow denominator matmul
        p8_t = [sb.tile([128, SCHED[L], QC], DT_F8, tag=f"p8_{L}", name=f"p8_{L}")
                for L in range(4)]
        px_t = {}
        for L in range(4):
            for i in range(DT8):
                px_t[(L, i)] = sb.tile([128, QC], DT_BF, tag=f"px{L}_{i}",
                                       name=f"px{L}_{i}")

        def p_slice(L, b):
            t = p01_t if L < 2 else p23_t
            return t[b][:, QC * (L % 2) : QC * (L % 2 + 1)]

        # Input DMAs ride sync+gpsimd only (scalar/vector run the
        # latency-critical softmax pipeline; an engine's in-order queue would
        # stall it behind queued DMA triggers), in consumption order so the
        # first matmuls start early.
        _dmaq = [nc.sync, nc.gpsimd]

        def _dma(n, dst, src):
            _dmaq[n % 2].dma_start(out=dst, in_=src)

        def dma_m(ot):
            for i in range(DT8):
                _dma(i + ot, m_t[i][ot],
                     m[128 * i : 128 * (i + 1), 128 * ot : 128 * (ot + 1)])

        def dma_xTq(h):
            for i in range(DT8):
                _dma(i + h, xTq_t[i][h],
                     xTq[128 * i : 128 * (i + 1), CH * h : CH * (h + 1)])

        def dma_xT(sc):
            for i in range(DT8):
                _dma(i + sc, xT_t[i][sc],
                     xT[128 * i : 128 * (i + 1), CH * sc : CH * (sc + 1)])

        def dma_xS(b0, b1):
            for b in range(b0, b1):
                _dma(b, xS_t[b], xS[128 * b : 128 * (b + 1), :])

        def dma_msk(L):
            for b in range(SCHED[L]):
                _dma(b, msk_t[MASK_BASE[L] + b], msk[MASK_BASE[L] + b])

        def dma_wv():
            for i in range(DT8):
                _dma(i + 1, wv_t[i], wvT[128 * i : 128 * (i + 1), :])

        dma_m(0)
        dma_xTq(0)
        for ot in range(1, DT8):
            dma_m(ot)
        dma_xT(0)
        dma_msk(0)
        dma_xT(1)
        dma_msk(1)
        dma_xTq(1)
        dma_xS(0, 8)
        dma_xT(2)
        dma_msk(2)
        dma_wv()
        dma_xS(8, 12)
        dma_xT(3)
        dma_msk(3)
        dma_xS(12, 16)

        # ---- PE pipeline ----
        def q_proj(h):
            # q'T[j, q] for the L-pair (2h, 2h+1): 512-wide rhs covers both
            for ot in range(DT8):
                ps = qps.tile([128, CH], DT_F32, tag="qps", name="qps")
                for i in range(DT8):
                    nc.tensor.matmul(
                        ps,
                        lhsT=m_t[i][ot],
                        rhs=xTq_t[i][h],
                        start=(i == 0),
                        stop=(i == DT8 - 1),
                    )
                nc.scalar.copy(out=qT_t[ot][:, CH * h : CH * (h + 1)], in_=ps)

        def scores(L):
            for b in range(SCHED[L]):
                ps = sps.tile([128, QC], DT_F32, tag="sps", name="sps")
                for i in range(DT8):
                    nc.tensor.matmul(
                        ps,
                        lhsT=xT_t[i][b // 4][:, BLK * (b % 4) : BLK * (b % 4 + 1)],
                        rhs=qT_t[i][:, QC * L : QC * (L + 1)],
                        start=(i == 0),
                        stop=(i == DT8 - 1),
                    )
                es = esb.tile([128, QC], DT_BF, tag="es", name="es")
                nc.scalar.activation(es, ps, Exp)
                mk = msk_t[MASK_BASE[L] + b]
                nc.vector.tensor_mul(p_slice(L, b), es, mk)
                if L > 0:
                    nc.vector.tensor_mul(p8_t[L][:, b, :], es, mk)

        def px(Le):
            # PxT[i, q] for the L-pair (Le, Le+1): one matmul covers both
            # chunks' q-columns while b < SCHED[Le]; beyond that only the
            # odd chunk's half accumulates.
            nb_e, nb_o = SCHED[Le], SCHED[Le + 1]
            pt = p01_t if Le == 0 else p23_t
            for i in range(DT8):
                ps = pxps.tile([128, 2 * QC], DT_F32, tag="pxps", name="pxps")
                for b in range(nb_o):
                    w = xS_t[b][:, 128 * i : 128 * (i + 1)]
                    if b < nb_e:
                        nc.tensor.matmul(
                            ps, lhsT=w, rhs=pt[b],
                            start=(b == 0), stop=(b == nb_o - 1),
                            skip_group_check=True,
                        )
                    else:
                        nc.tensor.matmul(
                            ps[:, QC : 2 * QC], lhsT=w, rhs=pt[b][:, QC : 2 * QC],
                            start=False, stop=(b == nb_o - 1),
                            skip_group_check=True,
                        )
                if i % 2 == 0:
                    nc.scalar.copy(out=px_t[(Le, i)], in_=ps[:, 0:QC])
                    nc.vector.tensor_copy(out=px_t[(Le + 1, i)], in_=ps[:, QC : 2 * QC])
                else:
                    nc.vector.tensor_copy(out=px_t[(Le, i)], in_=ps[:, 0:QC])
                    nc.scalar.copy(out=px_t[(Le + 1, i)], in_=ps[:, QC : 2 * QC])

        def den_out(L):
            for sqt in range(QC // 128):
                pd = sps.tile([128, QC], DT_F32, tag="sps", name="sps")[:, 0:1]
                if L == 0:
                    # exact bf16 den for the shortest chunk: its first rows
                    # see too few keys for the fp8 den error to average out
                    for b in range(SCHED[0]):
                        nc.tensor.matmul(
                            pd,
                            lhsT=p_slice(0, b)[:, 128 * sqt : 128 * (sqt + 1)],
                            rhs=ones,
                            start=(b == 0),
                            stop=(b == SCHED[0] - 1),
                        )
                else:
                    for u in range(SCHED[L] // 2):
                        nc.tensor.matmul(
                            pd,
                            lhsT=p8_t[L][:, 2 * u : 2 * u + 2,
                                         128 * sqt : 128 * (sqt + 1)],
                            rhs=ones8,
                            perf_mode=DR,
                            start=(u == 0),
                            stop=(u == SCHED[L] // 2 - 1),
                        )
                po = ops.tile([128, D], DT_F32, tag="po", name="po")
                for i in range(DT8):
                    pxs = px_t[(L, i)][:, 128 * sqt : 128 * (sqt + 1)]
                    nc.tensor.matmul(
                        po[:, 0:CH], lhsT=pxs, rhs=wv_t[i][:, 0:CH],
                        start=(i == 0), stop=(i == DT8 - 1),
                        skip_group_check=True,
                    )
                    nc.tensor.matmul(
                        po[:, CH:D], lhsT=pxs, rhs=wv_t[i][:, CH:D],
                        start=(i == 0), stop=(i == DT8 - 1),
                        skip_group_check=True,
                    )
                r = osb.tile([128, 1], DT_F32, tag="r", name="r")
                nc.vector.reciprocal(r, pd)
                o = osb.tile([128, D], DT_BF, tag="osb", name="osb")
                if (L + sqt) % 2 == 0:
                    nc.scalar.activation(o, po, Copy, scale=r)
                else:
                    nc.vector.tensor_scalar_mul(o, po, r)
                nc.sync.dma_start(
                    out=out[QC * L + 128 * sqt : QC * L + 128 * (sqt + 1), :],
                    in_=o,
                )

        q_proj(0)
        scores(0)
        scores(1)
        q_proj(1)
        scores(2)
        px(0)
        den_out(0)
        den_out(1)
        scores(3)
        px(2)
        den_out(2)
        den_out(3)


def build_program():
    nc = bacc.Bacc(
        "TRN2",
        target_bir_lowering=False,
        debug=False,
        enable_asserts=False,
        num_devices=N_CORES,
    )
    xT = nc.dram_tensor("xT", [D, S], DT_BF, kind="ExternalInput").ap()
    xTq = nc.dram_tensor("xTq", [D, 4 * QC], DT_BF, kind="ExternalInput").ap()
    xS = nc.dram_tensor("xS", [S, D], DT_BF, kind="ExternalInput").ap()
    m = nc.dram_tensor("m", [D, D], DT_BF, kind="ExternalInput").ap()
    wvT = nc.dram_tensor("wvT", [D, D], DT_BF, kind="ExternalInput").ap()
    msk = nc.dram_tensor("msk", [NMASK, BLK, QC], DT_BF, kind="ExternalInput").ap()
    out = nc.dram_tensor("out", [4 * QC, D], DT_BF, kind="ExternalOutput").ap()
    with tile.TileContext(nc) as tc:
        _emit(tc, xT, xTq, xS, m, wvT, msk, out)
    nc.compile()
    return nc


def get_program():
    if "nc" not in _NC_CACHE:
        _NC_CACHE["nc"] = build_program()
    return _NC_CACHE["nc"]


def _chunks_for(core):
    """Per-core 256-wide query chunks, L-ordered to match SCHED=(4,8,12,16).
    Real causal k-block need: chunk j -> 2(j+1)."""
    return [0, 3, 4, 7] if core % 2 == 0 else [1, 2, 5, 6]


def _build_masks(chunks, permuted):
    """[40,128,256] in {0,1}: allowed iff actual_key <= actual_query, where
    for odd cores the key axis is permuted by pos^256 (see build_in_maps).
    Padding blocks beyond a chunk's real causal depth come out all-zero."""
    m = np.zeros((NMASK, BLK, QC), np.float32)
    p = np.arange(BLK)[:, None]
    c = np.arange(QC)[None, :]
    for L, j in enumerate(chunks):
        for b in range(SCHED[L]):
            sk = BLK * b + p
            if permuted:
                sk = sk ^ 256
            m[MASK_BASE[L] + b] = sk <= QC * j + c
    return m.astype(bf16)


def _perm256(a, axis):
    """Swap the 256-halves of every 512-chunk along `axis` (pos -> pos^256)."""
    sh = a.shape
    n = sh[axis]
    new_shape = sh[:axis] + (n // 512, 2, 256) + sh[axis + 1 :]
    return np.ascontiguousarray(
        np.flip(a.reshape(new_shape), axis=axis + 1).reshape(sh)
    )


def build_in_maps(x, Wq, Wk, Wv):
    Wq = np.asarray(Wq, np.float32)
    Wk = np.asarray(Wk, np.float32)
    Wv = np.asarray(Wv, np.float32)
    m = ((Wq.T @ Wk) / 32.0).astype(bf16)  # [d_in, d_in], softmax scale folded
    wv = np.ascontiguousarray(Wv.T).astype(bf16)
    masks = {par: _build_masks(_chunks_for(par), par == 1) for par in (0, 1)}
    in_maps = []
    for core in range(N_CORES):
        b = core // 2
        xb = np.asarray(x[b], np.float32).astype(bf16)  # [S, D]
        xT = np.ascontiguousarray(np.asarray(x[b], np.float32).T)
        xq = np.concatenate(
            [xT[:, QC * j : QC * (j + 1)] for j in _chunks_for(core)], axis=1
        ).astype(bf16)
        if core % 2 == 1:
            # Key/seq-permute by pos^256 so both parities share one
            # instruction stream; xT columns, xS rows and mask key
            # coordinates move together (q-side xTq is gathered on host and
            # needs no permutation).
            xT = _perm256(xT, 1)
            xb = _perm256(xb, 0)
        in_maps.append(
            {"xT": xT.astype(bf16), "xTq": xq, "xS": xb, "m": m, "wvT": wv,
             "msk": masks[core % 2]}
        )
    return in_maps


def assemble_output(results):
    out = np.zeros((B, S, D), np.float32)
    for core in range(N_CORES):
        b = core // 2
        for L, j in enumerate(_chunks_for(core)):
            out[b, QC * j : QC * (j + 1)] = \
                results[core]["out"][QC * L : QC * (L + 1)].astype(np.float32)
    return out


def kernel(x, Wq, Wk, Wv):
    x = np.asarray(x, np.float32)
    nc = get_program()
    in_maps = build_in_maps(x, np.asarray(Wq, np.float32),
                            np.asarray(Wk, np.float32), np.asarray(Wv, np.float32))
    res = run_bass_kernel_spmd(nc, in_maps, core_ids=list(range(N_CORES)))
    return assemble_output(res.results)
